# revision 7
# baseline (speedup 1.0000x reference)
"""Trainium2 Bass kernel for nn_DenseIouPred.

The reference module computes, for sample 0 only, a dense (72, 72) IoU map:
for every offset (dh, dw) in a (2r+1)^2 window around the center decoded from
`ind`, it gathers the predicted ltrb box at map position (ch+dh, cw+dw),
compares it with the target box shifted by the offset, and scatters the IoU
to that same map position.  Because the gathered index equals the scattered
index, the whole computation is a dense elementwise map over the 72x72 grid
with a separable (row x col) validity mask:

  out[r, c] = vr[r] * vc[c] * (A + 1) / (T + P - A + 1)
    A = (min(pl, twl[c]) + min(pr, twr[c])) * (min(pb, thb[r]) + min(pt, tht[r]))
    P = (pl + pr) * (pt + pb)          # pl..pb = output[0,0,:,r,c]
    twl[c] = t0 + (c - cw),  twr[c] = t1 - (c - cw)
    tht[r] = t2 + (r - ch),  thb[r] = t3 - (r - ch)
    T = (t0 + t1) * (t2 + t3)
    vc[c] = (|c - cw| <= radius) & (twl[c] >= 0) & (twr[c] >= 0)
    vr[r] = (|r - ch| <= radius) & (tht[r] >= 0) & (thb[r] >= 0)

Host prep is O(72): decode the center and build seven 72-vectors.  The O(W*H)
math runs on device.  All 8 cores run the same tiny kernel (SPMD, replicated
inputs); core 0's output is returned.
"""

import numpy as np

W = 72  # map width == height
DIM = 4

_NC_CACHE = {}
LAST_RESULT = None  # BassKernelResults of the most recent run (for profiling)


def _build_nc():
    import concourse.bacc as bacc
    import concourse.tile as tile
    from concourse import mybir

    Op = mybir.AluOpType
    f32 = mybir.dt.float32

    nc = bacc.Bacc(None, target_bir_lowering=False, name="dense_iou_pred")
    pred_d = nc.dram_tensor("pred", [DIM, W, W], f32, kind="ExternalInput")
    # hv rows: 0 twl[c], 1 twr[c], 2 vc[c], 3 tht[r], 4 thb[r], 5 vr[r], 6 T+1
    hv_d = nc.dram_tensor("hv", [7, W], f32, kind="ExternalInput")
    out_d = nc.dram_tensor("iou_map", [W, W], f32, kind="ExternalOutput")

    with tile.TileContext(nc) as tc:
        with tc.tile_pool(name="pool", bufs=1) as pool:
            planes = pool.tile([W, DIM, W], f32, tag="planes")
            pl = planes[:, 0, :]
            pr = planes[:, 1, :]
            pt = planes[:, 2, :]
            pb = planes[:, 3, :]
            for chn in range(DIM):
                nc.sync.dma_start(out=planes[:, chn, :], in_=pred_d[chn])

            twl_b = pool.tile([W, W], f32, tag="twl")  # col vectors broadcast
            twr_b = pool.tile([W, W], f32, tag="twr")  # across all partitions
            vc_b = pool.tile([W, W], f32, tag="vc")
            nc.sync.dma_start(out=twl_b, in_=hv_d[0:1, :].to_broadcast((W, W)))
            nc.sync.dma_start(out=twr_b, in_=hv_d[1:2, :].to_broadcast((W, W)))
            nc.sync.dma_start(out=vc_b, in_=hv_d[2:3, :].to_broadcast((W, W)))

            percol = pool.tile([W, 4], f32, tag="percol")  # [tht, thb, vr, T+1]
            nc.sync.dma_start(out=percol, in_=hv_d[3:7, :].rearrange("a b -> b a"))
            tht_s = percol[:, 0:1]
            thb_s = percol[:, 1:2]
            vr_s = percol[:, 2:3]
            ta1_s = percol[:, 3:4]

            res = pool.tile([W, W], f32, tag="res")
            nc.gpsimd.memset(res, 0.0)

            w1 = pool.tile([W, W], f32, tag="w1")
            w2 = pool.tile([W, W], f32, tag="w2")
            h1 = pool.tile([W, W], f32, tag="h1")
            h2 = pool.tile([W, W], f32, tag="h2")
            slr = pool.tile([W, W], f32, tag="slr")
            stb = pool.tile([W, W], f32, tag="stb")
            parea = pool.tile([W, W], f32, tag="parea")
            mask = pool.tile([W, W], mybir.dt.uint8, tag="mask")
            den = pool.tile([W, W], f32, tag="den")
            rec = pool.tile([W, W], f32, tag="rec")
            iou = pool.tile([W, W], f32, tag="iou")

            # pred_area = (pl + pr) * (pt + pb) and the mask go on GpSimd,
            # off the DVE critical path.
            nc.gpsimd.tensor_tensor(out=slr, in0=pl, in1=pr, op=Op.add)
            nc.gpsimd.tensor_tensor(out=stb, in0=pt, in1=pb, op=Op.add)
            nc.gpsimd.tensor_tensor(out=parea, in0=slr, in1=stb, op=Op.mult)
            nc.vector.tensor_scalar_mul(out=mask, in0=vc_b, scalar1=vr_s)

            # intersection area A
            nc.vector.tensor_tensor(out=w1, in0=pl, in1=twl_b, op=Op.min)
            nc.vector.tensor_tensor(out=w2, in0=pr, in1=twr_b, op=Op.min)
            nc.vector.tensor_scalar_min(out=h1, in0=pb, scalar1=thb_s)
            nc.vector.tensor_scalar_min(out=h2, in0=pt, scalar1=tht_s)
            nc.vector.tensor_tensor(out=w1, in0=w1, in1=w2, op=Op.add)
            nc.vector.tensor_tensor(out=h1, in0=h1, in1=h2, op=Op.add)
            nc.vector.tensor_tensor(out=w1, in0=w1, in1=h1, op=Op.mult)  # A

            # den = (P + (T+1)) - A ; iou = (A + 1) * (1 / den)
            nc.vector.scalar_tensor_tensor(
                out=den, in0=parea, scalar=ta1_s, in1=w1, op0=Op.add, op1=Op.subtract
            )
            nc.vector.reciprocal(out=rec, in_=den)
            nc.vector.scalar_tensor_tensor(
                out=iou, in0=w1, scalar=1.0, in1=rec, op0=Op.add, op1=Op.mult
            )
            nc.vector.copy_predicated(out=res, mask=mask, data=iou)

            nc.sync.dma_start(out=out_d[:, :], in_=res)
    nc.finalize()
    return nc


def _host_prep(output, ind, target, radius):
    out0 = np.ascontiguousarray(
        np.asarray(output).reshape(-1, DIM, W, W)[0], dtype=np.float32
    )
    t = np.asarray(target).reshape(-1, DIM)[0].astype(np.float32)
    i0 = int(np.asarray(ind).reshape(-1)[0])
    r = float(int(radius) if np.ndim(radius) == 0 else int(np.asarray(radius)))
    cw = np.float32(i0 % W)
    ch = np.float32(i0 // W)

    idx = np.arange(W, dtype=np.float32)
    rw = idx - cw
    rh = idx - ch
    twl = t[0] + rw
    twr = t[1] - rw
    tht = t[2] + rh
    thb = t[3] - rh
    vc = ((np.abs(rw) <= r) & (twl >= 0) & (twr >= 0)).astype(np.float32)
    vr = ((np.abs(rh) <= r) & (tht >= 0) & (thb >= 0)).astype(np.float32)
    ta1 = np.float32(t[0] + t[1]) * np.float32(t[2] + t[3]) + np.float32(1.0)
    hv = np.stack([twl, twr, vc, tht, thb, vr, np.full(W, ta1, np.float32)])
    return out0, np.ascontiguousarray(hv, dtype=np.float32)


def kernel(output, ind, target, radius):
    global LAST_RESULT
    from concourse.bass_utils import run_bass_kernel_spmd

    out0, hv = _host_prep(output, ind, target, radius)

    if "nc" not in _NC_CACHE:
        _NC_CACHE["nc"] = _build_nc()
    nc = _NC_CACHE["nc"]

    in_map = {"pred": out0, "hv": hv}
    n_cores = 8
    res = run_bass_kernel_spmd(nc, [in_map] * n_cores, core_ids=list(range(n_cores)))
    LAST_RESULT = res
    return np.asarray(res.results[0]["iou_map"], dtype=np.float32)


# revision 12
# speedup vs baseline: 1.3950x; 1.3950x over previous
"""Trainium2 Bass kernel for nn_DenseIouPred.

The reference module computes, for sample 0 only, a dense (72, 72) IoU map:
for every offset (dh, dw) in a (2r+1)^2 window around the center decoded from
`ind`, it gathers the predicted ltrb box at map position (ch+dh, cw+dw),
compares it with the target box shifted by the offset, and scatters the IoU to
that same map position.  Because the gathered index equals the scattered index,
the whole computation is a dense elementwise map over the 72x72 grid with a
separable (row x col) validity mask:

  out[r, c] = vr[r] * vc[c] * (A + 1) / (T + P - A + 1)
    A = (min(pl, twl[c]) + min(pr, twr[c])) * (min(pb, thb[r]) + min(pt, tht[r]))
    P = (pl + pr) * (pt + pb)          # pl..pb = output[0,0,:,r,c]
    twl[c] = t0 + (c - cw),  twr[c] = t1 - (c - cw)
    tht[r] = t2 + (r - ch),  thb[r] = t3 - (r - ch)
    T = (t0 + t1) * (t2 + t3)
    vc[c] = (|c - cw| <= radius) & (twl[c] >= 0) & (twr[c] >= 0)
    vr[r] = (|r - ch| <= radius) & (tht[r] >= 0) & (thb[r] >= 0)

Host prep is O(W^2) numpy packing: one (72, 649) buffer whose row r holds
[pl[r]|pr[r]|pt[r]|pb[r] | twl|twr|tht[r]*1|thb[r]*1 | mask[r] | T+1].  The
device kernel is a raw Bacc program: two parallel input DMAs (SP + Activation
HWDGE queues), seven chained DVE ops (channel pairs fused via strided access
patterns), one output DMA.  All 8 cores run the same tiny kernel (SPMD,
replicated inputs); core 0's output is returned.

SBUF free-dim layout (fp32 words, one 72-partition tensor):
  0:288    planes [pl|pr|pt|pb]
  288:576  limits [twl|twr|tht|thb]
  576:648  mask (fp32 0/1)
  648:649  T+1
  652:940  M = min(planes, limits)
  940:1228 V = [pl+pr | pt+pb | mL+mR | mT+mB]    (one fused add)
  1228:1372 R = [P | A]                            (one fused mul)
  1372:1444 den = (P + (T+1)) - A
  1444:1516 rec ~= 1/den
  1516:1588 iou = (A+1)*rec
  1588:1660 res = iou * mask
"""

import numpy as np

W = 72
DIM = 4

# fp32-word offsets in the SBUF scratch tensor
_PLANES = 0
_LIMITS = 288
_MASK = 576
_TA1 = 648
_M = 652
_V = 940
_R = 1228
_DEN = 1372
_REC = 1444
_IOU = 1516
_RES = 1588
_HBW = 1660  # total free words
_NIN = 649  # DRAM input row words
_SPLIT = 324  # DMA split point across the two HWDGE queues

_NC_CACHE = {}
LAST_RESULT = None


def _build_nc():
    import concourse.bacc as bacc
    import concourse.bass as bass
    from concourse import mybir

    Op = mybir.AluOpType
    f32 = mybir.dt.float32
    AP = bass.AP

    nc = bacc.Bacc(
        None,
        target_bir_lowering=False,
        enable_partition_id=False,
        name="dense_iou_pred",
    )
    hb_d = nc.dram_tensor("hb", [W, _NIN], f32, kind="ExternalInput")
    out_d = nc.dram_tensor("iou_map", [W, W], f32, kind="ExternalOutput")

    with (
        nc.semaphore("in_sem") as in_sem,
        nc.semaphore("v_sem") as v_sem,
        nc.sbuf_tensor("sb_hb", [W, _HBW], f32) as hb,
        nc.Block() as block,
    ):
        def sb(off, pattern):
            return AP(hb, off, [[_HBW, W]] + pattern)

        @block.sync
        def _(sync):
            sync.dma_start(
                AP(hb, 0, [[_HBW, W], [1, _SPLIT]]),
                hb_d[:, 0:_SPLIT],
            ).then_inc(in_sem, 16)
            sync.wait_ge(v_sem, 1)
            sync.dma_start(out_d[:, :], sb(_RES, [[1, W]])).then_inc(in_sem, 16)
            sync.wait_ge(in_sem, 48)

        @block.scalar
        def _(scalar):
            scalar.dma_start(
                AP(hb, _SPLIT, [[_HBW, W], [1, _NIN - _SPLIT]]),
                hb_d[:, _SPLIT:_NIN],
            ).then_inc(in_sem, 16)

        @block.vector
        def _(vector):
            ch4 = [[W, DIM], [1, W]]
            vector.wait_ge(in_sem, 32)
            # M = min(planes, limits): all 4 channel pairs in one op
            vector.tensor_tensor(
                out=sb(_M, ch4), in0=sb(_PLANES, ch4), in1=sb(_LIMITS, ch4), op=Op.min
            )
            # V = [pl+pr, pt+pb, mL+mR, mT+mB] in one op
            pairs_in = [[_M - _PLANES, 2], [2 * W, 2], [1, W]]
            vector.tensor_tensor(
                out=sb(_V, [[2 * W, 2], [W, 2], [1, W]]),
                in0=sb(_PLANES, pairs_in),
                in1=sb(_PLANES + W, pairs_in),
                op=Op.add,
            )
            # R = [P, A] = [slr*stb, wsum*hsum] in one op
            two = [[2 * W, 2], [1, W]]
            vector.tensor_tensor(
                out=sb(_R, [[W, 2], [1, W]]),
                in0=sb(_V, two),
                in1=sb(_V + W, two),
                op=Op.mult,
            )
            one = [[1, W]]
            # den = (P + (T+1)) - A
            vector.scalar_tensor_tensor(
                out=sb(_DEN, one),
                in0=sb(_R, one),
                scalar=sb(_TA1, [[1, 1]]),
                in1=sb(_R + W, one),
                op0=Op.add,
                op1=Op.subtract,
            )
            vector.reciprocal_approx_fast(out=sb(_REC, one), in_=sb(_DEN, one))
            # iou = (A + 1) * rec
            vector.scalar_tensor_tensor(
                out=sb(_IOU, one),
                in0=sb(_R + W, one),
                scalar=1.0,
                in1=sb(_REC, one),
                op0=Op.add,
                op1=Op.mult,
            )
            vector.tensor_tensor(
                out=sb(_RES, one), in0=sb(_IOU, one), in1=sb(_MASK, one), op=Op.mult
            ).then_inc(v_sem, 1)

    nc.finalize()
    return nc


def _host_prep(output, ind, target, radius):
    out0 = np.asarray(output).reshape(-1, DIM, W, W)[0].astype(np.float32)
    t = np.asarray(target).reshape(-1, DIM)[0].astype(np.float32)
    i0 = int(np.asarray(ind).reshape(-1)[0])
    r = float(int(np.asarray(radius)))
    cw = np.float32(i0 % W)
    ch = np.float32(i0 // W)

    idx = np.arange(W, dtype=np.float32)
    rw = idx - cw
    rh = idx - ch
    twl = t[0] + rw
    twr = t[1] - rw
    tht = t[2] + rh
    thb = t[3] - rh
    vc = ((np.abs(rw) <= r) & (twl >= 0) & (twr >= 0)).astype(np.float32)
    vr = ((np.abs(rh) <= r) & (tht >= 0) & (thb >= 0)).astype(np.float32)
    ta1 = np.float32(t[0] + t[1]) * np.float32(t[2] + t[3]) + np.float32(1.0)

    hb = np.empty((W, _NIN), dtype=np.float32)
    hb[:, 0:288] = out0.transpose(1, 0, 2).reshape(W, DIM * W)
    hb[:, 288:360] = twl[None, :]
    hb[:, 360:432] = twr[None, :]
    hb[:, 432:504] = tht[:, None]
    hb[:, 504:576] = thb[:, None]
    hb[:, 576:648] = vr[:, None] * vc[None, :]
    hb[:, 648] = ta1
    return np.ascontiguousarray(hb)


def kernel(output, ind, target, radius):
    global LAST_RESULT
    from concourse.bass_utils import run_bass_kernel_spmd

    hb = _host_prep(output, ind, target, radius)

    if "nc" not in _NC_CACHE:
        _NC_CACHE["nc"] = _build_nc()
    nc = _NC_CACHE["nc"]

    in_map = {"hb": hb}
    n_cores = 8
    res = run_bass_kernel_spmd(nc, [in_map] * n_cores, core_ids=list(range(n_cores)))
    LAST_RESULT = res
    return np.asarray(res.results[0]["iou_map"], dtype=np.float32)
